# revision 32
# baseline (speedup 1.0000x reference)
"""BiMambaBlock on 8 Trainium2 NeuronCores — v3.

Core c = (batch b, direction d, d_inner-half h), as v2.

v2 trace (585.8us): DVE busy 488us of span (64 scans x 4.41us = 282,
160 TTs = 194, conv TS = 24); prologue 120us bounded by the serial ACT
queue (drains/silus/seeds before the first dA exp) and the PE in_proj
chain; tail 45us of un-overlapped out-proj.

v3 keeps the DVE stream identical (scan+dBx+p are irreducible on DVE)
and attacks prologue + tail:

  - y accumulation for ALL blocks via gpsimd DMA-accum (no PE identity
    matmuls, no PSUM y banks, no tail PSUM drains).
  - n-loop split into pass A (blocks 0,1) and pass B (blocks 2,3):
    after pass A, blocks 0/1 are gated and their out-proj contribution
    (32 mm + drains + DMA to a separate DRAM output out_partA) runs
    UNDER pass B; host adds the two partials. Tail = pass-B epilogue
    only.
  - z-gate drains fused with silu (gz = silu(z_psum)): phase-3 silu
    ops gone, and phase-2 ACT is pure Exp+Copy (no table thrash).
  - y D-seed moved off the prologue ACT queue: seeded via a deferred
    DMA-accum inside the pass (scalar.mul runs in pass slack).
  - in_proj PSUM drains alternate ACT/DVE to shorten the serial ACT
    prologue (DVE has prologue slack).
  - out-proj-A drains are emitted after pass B's first iterations so
    pass B's dA exps aren't queued behind them on ACT.

GPSIMD compute stays idle (Pool TT degrades DVE 2x TTs 4x / scans 2x,
measured in a prior session); only its software-DGE DMA queue is used.
"""
import os
import sys

for _p in ("/opt/trn_rl_repo",):
    if os.path.isdir(_p) and _p not in sys.path:
        sys.path.insert(0, _p)

from contextlib import ExitStack

import ml_dtypes
import numpy as np

from concourse import bass, mybir, tile
from concourse.bass_utils import run_bass_kernel_spmd

F32 = mybir.dt.float32
BF16 = mybir.dt.bfloat16
AF = mybir.ActivationFunctionType
OP = mybir.AluOpType

D_MODEL = 512
D_INNER = 1024
DH = 512
N_STATE = 16
D_CONV = 4
DT_RANK = 32
B = 2
L = 2048
LP = L + 3

NBLK_F = D_INNER // 128  # 8 channel blocks for conv/xproj
NBLK_H = DH // 128       # 4 scan blocks

NCH = L // 512           # 512-wide matmul N-chunks

BF16NP = ml_dtypes.bfloat16


def _build_program():
    nc = bass.Bass(trn_type="TRN2", target_bir_lowering=False, debug=False)

    xT_d = nc.dram_tensor("xT", [128, 4 * LP], BF16, kind="ExternalInput")
    w_in_d = nc.dram_tensor("w_in", [128, 4 * 1536], BF16, kind="ExternalInput")
    conv_w_d = nc.dram_tensor("conv_w", [128, NBLK_F * D_CONV], F32, kind="ExternalInput")
    conv_b_d = nc.dram_tensor("conv_b", [128, NBLK_F], F32, kind="ExternalInput")
    xproj_w_d = nc.dram_tensor("xproj_w", [128, NBLK_F * 64], BF16, kind="ExternalInput")
    dt_w_d = nc.dram_tensor("dt_w", [DT_RANK, DH], BF16, kind="ExternalInput")
    dt_b_d = nc.dram_tensor("dt_b", [128, NBLK_H], F32, kind="ExternalInput")
    A_d = nc.dram_tensor("A", [128, NBLK_H * N_STATE], F32, kind="ExternalInput")
    D_d = nc.dram_tensor("D", [128, NBLK_H], F32, kind="ExternalInput")
    w_out_d = nc.dram_tensor("w_out", [128, 4 * D_MODEL], BF16, kind="ExternalInput")
    outA_d = nc.dram_tensor("out_partA", [D_MODEL, L], F32, kind="ExternalOutput")
    outB_d = nc.dram_tensor("out_partB", [D_MODEL, L], F32, kind="ExternalOutput")

    with tile.TileContext(nc) as tc, ExitStack() as ctx:
        # ---------------- global pools / persistent tiles ----------------
        wp = ctx.enter_context(tc.tile_pool(name="weights", bufs=1))

        # per-kb tiles so the first matmul waits on one DMA pair, not
        # all four writes into a shared tile
        xT_k = [wp.tile([128, LP], BF16, tag=f"xT{kb}", name=f"xT{kb}")
                for kb in range(4)]
        w_in_k = [wp.tile([128, 1536], BF16, tag=f"w_in{kb}",
                          name=f"w_in{kb}") for kb in range(4)]
        conv_w = wp.tile([128, NBLK_F * D_CONV], F32, tag="conv_w", name="conv_w")
        conv_b = wp.tile([128, NBLK_F], F32, tag="conv_b", name="conv_b")
        xproj_w = wp.tile([128, NBLK_F * 64], BF16, tag="xproj_w", name="xproj_w")
        dt_w = wp.tile([DT_RANK, DH], BF16, tag="dt_w", name="dt_w")
        dt_b = wp.tile([128, NBLK_H], F32, tag="dt_b", name="dt_b")
        A_sb = wp.tile([128, NBLK_H * N_STATE], F32, tag="A", name="A_sb")
        D_sb = wp.tile([128, NBLK_H], F32, tag="D", name="D_sb")
        w_out = wp.tile([128, 4 * D_MODEL], BF16, tag="w_out", name="w_out")

        for kb in range(4):
            nc.sync.dma_start(w_in_k[kb][:],
                              w_in_d[:, kb * 1536:(kb + 1) * 1536])
            nc.sync.dma_start(xT_k[kb][:],
                              xT_d[:, kb * LP:(kb + 1) * LP])
            if kb == 0:
                nc.sync.dma_start(conv_w[:], conv_w_d[:])
                nc.sync.dma_start(conv_b[:], conv_b_d[:])
        for t, d in [(xproj_w, xproj_w_d), (dt_w, dt_w_d),
                     (dt_b, dt_b_d), (A_sb, A_d), (D_sb, D_d), (w_out, w_out_d)]:
            nc.sync.dma_start(t[:], d[:])

        def xT_v(kb, sl):
            return xT_k[kb][:, sl]

        def w_in_v(kb, sl):
            return w_in_k[kb][:, sl]

        xproj_v = xproj_w[:].rearrange("p (k f) -> p k f", k=NBLK_F)
        w_out_v = w_out[:].rearrange("p (k m) -> p k m", k=4)

        glob = ctx.enter_context(tc.tile_pool(name="glob", bufs=1))
        xc_t = [glob.tile([128, L], BF16, tag=f"xc{i}", name=f"xc{i}")
                for i in range(NBLK_H)]  # own-half xc, live till the end
        dt_t = [glob.tile([128, L], BF16, tag=f"dt{i}", name=f"dt{i}")
                for i in range(NBLK_H)]
        dtx_t = [glob.tile([128, L], BF16, tag=f"dtx{i}", name=f"dtx{i}")
                 for i in range(NBLK_H)]
        y_t = [glob.tile([128, L], BF16, tag=f"y{i}", name=f"y{i}")
               for i in range(NBLK_H)]
        # gz reuses the xc tiles: xc's last read is the D-seed (early in
        # the scan loop), z's drains land iterations later — the WAR
        # dependency is tracked through the shared tile.
        gz_t = xc_t
        dbc = glob.tile([64, L], BF16, tag="dbc", name="dbc")

        def in_proj_block(m0, xi, xi_off, n_cols, pio):
            """matmul w_in cols [m0, m0+128) x xT -> xi[:, xi_off:...]

            kb-outer with all 5 chunk accumulators in flight: successive
            matmuls hit different PSUM tiles, avoiding the ~120-280ns
            read-modify-write hazard a serial accumulation chain pays
            (and keeping the PE ramped)."""
            chunks = list(range(0, n_cols, 512))
            pss = [pio.tile([128, 512], F32, tag="pio", name="ps_in")
                   for _ in chunks]
            for kb in range(4):
                for gi, nch in enumerate(chunks):
                    w = min(512, n_cols - nch)
                    nc.tensor.matmul(
                        pss[gi][:, 0:w],
                        lhsT=w_in_v(kb, slice(m0, m0 + 128)),
                        rhs=xT_v(kb, slice(nch, nch + w)),
                        start=(kb == 0), stop=(kb == 3),
                    )
            for gi, nch in enumerate(chunks):
                w = min(512, n_cols - nch)
                dst = xi[:, xi_off + nch:xi_off + nch + w]
                nc.scalar.copy(dst, pss[gi][:, 0:w])

        # pools for the scan phase. `pre_pool` (2 chain pairs + state-0
        # dA tiles) is created early so the prologue can pre-emit the
        # first broadcast chains and dA exps without reserving the full
        # scan-phase rings next to the phase-1 buffers.
        pre_pool = ctx.enter_context(tc.tile_pool(name="prechain", bufs=1))
        chains = {}
        dA_pre = {}
        chain_pool = None
        ph2 = None
        seedp = None

        def emit_chain(n, pool=None, sfx=""):
            pool = pool or chain_pool
            Bt = pool.tile([128, L], BF16, tag=f"Bt{sfx}", name="Bt")
            Ct = pool.tile([128, L], BF16, tag=f"Ct{sfx}", name="Ct")
            for ri, dst in ((32, Bt), (48, Ct)):
                nc.sync.dma_start(dst[0:1, :], dbc[ri + n:ri + n + 1, :])
                k = 1
                while k < 128:
                    nc.sync.dma_start(dst[k:2 * k, :], dst[0:k, :])
                    k *= 2
            return Bt, Ct

        def emit_dA(n, blk, pool=None, sfx=""):
            pool = pool or ph2
            dA = pool.tile([128, L], BF16, tag=f"dA{sfx}", name="dA")
            nc.scalar.activation(
                dA[:], dt_t[blk][:], AF.Exp,
                scale=A_sb[:, blk * N_STATE + n:blk * N_STATE + n + 1])
            return dA

        def emit_blk(n, blk, Bt, Ct, dA=None):
            if dA is None:
                dA = emit_dA(n, blk)
            dBx = ph2.tile([128, L], BF16, tag="dBx", name="dBx")
            nc.vector.tensor_tensor(dBx[:], dtx_t[blk][:], Bt[:], OP.mult)
            h = ph2.tile([128, L], BF16, tag="h", name="h")
            nc.vector.tensor_tensor_scan(
                h[:], dA[:], dBx[:], 0.0, OP.mult, OP.add)
            p = ph2.tile([128, L], BF16, tag="p", name="p")
            nc.vector.tensor_tensor(p[:], h[:], Ct[:], OP.mult)
            if n == 0:
                # first state: plain copy initializes y (no seed yet)
                nc.gpsimd.dma_start(y_t[blk][:], p[:])
            else:
                nc.gpsimd.dma_start(y_t[blk][:], p[:], accum_op=OP.add)

        def emit_seed(blk):
            # D_skip * xc, accumulated into y via the gpsimd DGE queue;
            # the scalar.mul runs in loop ACT slack, not the prologue.
            s = seedp.tile([128, L], BF16, tag="seed", name="seed")
            nc.scalar.mul(s[:], xc_t[blk][:], D_sb[:, blk:blk + 1])
            nc.gpsimd.dma_start(y_t[blk][:], s[:], accum_op=OP.add)

        def emit_gate(blk):
            # gz silu'd in the loop (batched, one table load)
            nc.vector.tensor_tensor(y_t[blk][:], y_t[blk][:], gz_t[blk][:],
                                    OP.mult)

        def emit_outproj(blks, out_d, pout, obp, ms, dve_drains=False):
            for m in ms:
                pss = [pout.tile([128, 512], F32, tag="pout", name="ps_out")
                       for _ in range(NCH)]
                for ki, kb in enumerate(blks):
                    for nch in range(NCH):
                        nc.tensor.matmul(
                            pss[nch][:],
                            lhsT=w_out_v[:, kb, m * 128:(m + 1) * 128],
                            rhs=y_t[kb][:, nch * 512:(nch + 1) * 512],
                            start=(ki == 0), stop=(ki == len(blks) - 1))
                for nch in range(NCH):
                    ob = obp.tile([128, 512], F32, tag="outb", name="outb")
                    if dve_drains and nch % 2 == 1:
                        # DVE is idle in the tail; split drains across it
                        nc.vector.tensor_copy(ob[:], pss[nch][:])
                    else:
                        nc.scalar.copy(ob[:], pss[nch][:])
                    nc.sync.dma_start(
                        out_d[m * 128:(m + 1) * 128,
                              nch * 512:(nch + 1) * 512], ob[:])

        def emit_z(blk, zp):
            # z-gate matmuls + raw copy drains, emitted late in the scan
            # loop: PE is idle there (and warm for the out-proj that
            # follows); silu applied in a separate batched pass. kb-outer
            # across all 4 chunk accumulators.
            pss = [zp.tile([128, 512], F32, tag="pz", name="ps_z")
                   for _ in range(NCH)]
            for kb in range(4):
                for nch in range(NCH):
                    nc.tensor.matmul(
                        pss[nch][:],
                        lhsT=w_in_v(kb, slice(1024 + blk * 128,
                                              1024 + (blk + 1) * 128)),
                        rhs=xT_v(kb, slice(3 + nch * 512,
                                           3 + (nch + 1) * 512)),
                        start=(kb == 0), stop=(kb == 3),
                    )
            for nch in range(NCH):
                nc.scalar.copy(gz_t[blk][:, nch * 512:(nch + 1) * 512],
                               pss[nch][:])

        # ---------------- phase 1: xc / xproj / dt ----------------
        with tc.tile_pool(name="ph1", bufs=1) as ph1, \
             tc.tile_pool(name="ph1b", bufs=2) as ph1b, \
             tc.tile_pool(name="pio", bufs=6, space="PSUM") as pio, \
             tc.tile_pool(name="pdbc", bufs=2, space="PSUM") as pdbc:
            for blk in range(NBLK_F):
                xi = ph1b.tile([128, LP], BF16, tag="xi", name="xi")
                in_proj_block(blk * 128, xi, 0, LP, pio)
                # conv taps all on DVE 4x tensor_scalar
                tk = []
                for k in range(D_CONV):
                    t = ph1b.tile([128, L], BF16, tag=f"ct{k}", name=f"ct{k}")
                    w_col = conv_w[:, blk * 4 + k:blk * 4 + k + 1]
                    nc.vector.tensor_scalar_mul(t[:], xi[:, k:k + L], w_col)
                    tk.append(t)
                nc.vector.tensor_tensor(tk[0][:], tk[0][:], tk[1][:], OP.add)
                nc.vector.tensor_tensor(tk[2][:], tk[2][:], tk[3][:], OP.add)
                acc = tk[0]
                nc.vector.tensor_tensor(acc[:], acc[:], tk[2][:], OP.add)
                if blk < NBLK_H:
                    xc = xc_t[blk]
                else:
                    xc = ph1.tile([128, L], BF16, tag=f"xcO{blk}",
                                  name=f"xcO{blk}")
                nc.scalar.activation(xc[:], acc[:], AF.Silu,
                                     bias=conv_b[:, blk:blk + 1])
                if blk < NBLK_H:
                    xc_t[blk] = xc
                else:
                    xc_t.append(xc)

            # xproj -> dbc.T [64, L]; chunk pairs interleaved to break
            # the 8-deep accumulation chains
            for nc0 in (0, 2):
                pss = [pdbc.tile([64, 512], F32, tag="pdbc", name="ps_dbc")
                       for _ in range(2)]
                for kb in range(NBLK_F):
                    for gi in range(2):
                        nch = nc0 + gi
                        nc.tensor.matmul(
                            pss[gi][:], lhsT=xproj_v[:, kb, :],
                            rhs=xc_t[kb][:, nch * 512:(nch + 1) * 512],
                            start=(kb == 0), stop=(kb == NBLK_F - 1),
                        )
                for gi in range(2):
                    nch = nc0 + gi
                    nc.scalar.copy(dbc[:, nch * 512:(nch + 1) * 512],
                                   pss[gi][:])

            # first two broadcast chains, issued as soon as dbc lands so
            # state 0 isn't chain-bound
            chains[0] = emit_chain(0, pre_pool, "0")
            chains[1] = emit_chain(1, pre_pool, "1")

            # dt = softplus(dt_raw.T + dt_b) = ln(1 + exp(.))
            for m in range(NBLK_H):
                dte = ph1b.tile([128, L], BF16, tag="dte", name="dte")
                for nch in range(NCH):
                    ps = pio.tile([128, 512], F32, tag="pio", name="ps_dt")
                    nc.tensor.matmul(
                        ps[:], lhsT=dt_w[:, m * 128:(m + 1) * 128],
                        rhs=dbc[0:DT_RANK, nch * 512:(nch + 1) * 512],
                        start=True, stop=True)
                    nc.scalar.activation(dte[:, nch * 512:(nch + 1) * 512],
                                         ps[:], AF.Exp, bias=dt_b[:, m:m + 1])
                nc.scalar.activation(dt_t[m][:], dte[:], AF.Ln, bias=1.0)
                if m < 2:
                    # dtx + state-0 dA for the skew-leading blocks only;
                    # blocks 2,3 don't scan until two iterations later,
                    # so their dtx TTs are emitted inside the loop and
                    # don't delay the first dBx on the DVE queue.
                    nc.vector.tensor_tensor(dtx_t[m][:], dt_t[m][:],
                                            xc_t[m][:], OP.mult)
                    dA_pre[m] = emit_dA(0, m, pre_pool, f"p{m}")

        # ---------------- phase 2: scan in two passes ----------------
        # ---------------- phase 2: skewed scan loop ----------------
        # Blocks 0,1 process state `it`; blocks 2,3 lag two states
        # (state `it-2`). Blocks 0,1 therefore finish two iterations
        # early and their gated out-proj (into out_partA) overlaps the
        # last two iterations; the tail is only blocks 2,3's epilogue.
        # One chain pair per iteration (v2's proven cadence), prefetched
        # two states ahead.
        chain_pool = ctx.enter_context(tc.tile_pool(name="chains", bufs=4))
        ph2 = ctx.enter_context(tc.tile_pool(name="ph2", bufs=2))
        seedp = ctx.enter_context(tc.tile_pool(name="seeds", bufs=2))
        with tc.tile_pool(name="pout", bufs=4, space="PSUM") as pout, \
             tc.tile_pool(name="pz", bufs=4, space="PSUM") as zp, \
             tc.tile_pool(name="obp", bufs=3) as obp:
            for it in range(N_STATE + 2):
                cn = it + 2
                if 2 <= cn < N_STATE:
                    chains[cn] = emit_chain(cn)
                if it < N_STATE:
                    s01 = it
                    emit_blk(s01, 0, *chains[s01], dA=dA_pre.pop(0, None))
                    emit_blk(s01, 1, *chains[s01], dA=dA_pre.pop(1, None))
                    if s01 == N_STATE - 1:
                        emit_gate(0)
                        emit_gate(1)
                if it < 2:
                    # deferred dtx for the lagging blocks (needed from
                    # iteration 2)
                    m = it + 2
                    nc.vector.tensor_tensor(dtx_t[m][:], dt_t[m][:],
                                            xc_t[m][:], OP.mult)
                s23 = it - 2
                if 0 <= s23 < N_STATE:
                    emit_blk(s23, 2, *chains[s23])
                    emit_blk(s23, 3, *chains[s23])
                    if s23 == N_STATE - 1:
                        emit_gate(2)
                        emit_gate(3)
                if it == 5:
                    emit_seed(0)
                    emit_seed(1)
                if it == 7:
                    emit_seed(2)
                    emit_seed(3)
                if 10 <= it <= 13:
                    emit_z(it - 10, zp)
                if it == 14:
                    nc.scalar.activation(gz_t[0][:], gz_t[0][:], AF.Silu)
                    nc.scalar.activation(gz_t[1][:], gz_t[1][:], AF.Silu)
                if it == 15:
                    nc.scalar.activation(gz_t[2][:], gz_t[2][:], AF.Silu)
                    nc.scalar.activation(gz_t[3][:], gz_t[3][:], AF.Silu)
                if it == 16:
                    emit_outproj((0, 1), outA_d, pout, obp, (0, 1))
                if it == 17:
                    emit_outproj((0, 1), outA_d, pout, obp, (2, 3))
            emit_outproj((2, 3), outB_d, pout, obp, range(4),
                         dve_drains=True)

    _split_excess_waits(nc)
    return nc


def _split_excess_waits(nc, max_waits=1):
    """The walrus build rejects instructions carrying more than one
    sync-wait command ("Too many sync wait commands" on Tile's kernel-tail
    Drain, which waits on every loose semaphore). Move excess waits onto
    NoOps placed just before the offender on the same engine."""
    for fn in nc.m.functions:
        for blk in fn.blocks:
            out, changed = [], False
            for inst in blk.instructions:
                si = inst.sync_info
                waits = list(si.on_wait) if si is not None and si.on_wait else []
                if len(waits) > max_waits:
                    extra, keep = waits[:-max_waits], waits[-max_waits:]
                    chunks = [extra[i:i + max_waits]
                              for i in range(0, len(extra), max_waits)]
                    for j, ch in enumerate(chunks):
                        nop = mybir.InstNoOp(
                            name=f"{inst.name}-waitsplit{j}", ins=[], outs=[])
                        nop.engine = inst.engine
                        nop.sync_info = mybir.SyncInfo(on_wait=ch, on_update=[])
                        out.append(nop)
                    si.on_wait = keep
                    changed = True
                out.append(inst)
            if changed:
                blk.instructions = out
    return nc


_PROG = None


def _get_program():
    global _PROG
    if _PROG is None:
        _PROG = _build_program()
    return _PROG


def _to_pblocks(a, nblk, dtype):
    """[nblk*128, f] -> [128, nblk*f] with [p, blk*f+j] = a[blk*128+p, j]."""
    a = np.ascontiguousarray(a)
    f = a.shape[1] if a.ndim > 1 else 1
    a = a.reshape(nblk, 128, f).transpose(1, 0, 2).reshape(128, nblk * f)
    return np.ascontiguousarray(a.astype(dtype))


def _core_inputs(hs, params, fuse_w, b, dr, h):
    p = params[dr]
    x = hs[b]
    if dr == 1:
        x = x[::-1]
    xTp = np.concatenate(
        [np.zeros((D_MODEL, 3), np.float32), np.ascontiguousarray(x.T)], axis=1)
    xT = _to_pblocks(xTp, 4, BF16NP)  # [128, 4*(L+3)] bf16

    sl_own = slice(h * DH, (h + 1) * DH)
    perm = np.r_[h * DH:(h + 1) * DH, (1 - h) * DH:(2 - h) * DH]

    in_w = p["in_w"]
    w_in_cols = np.concatenate(
        [in_w[:, :D_INNER][:, perm], in_w[:, D_INNER:][:, sl_own]], axis=1)
    w_in = _to_pblocks(w_in_cols, 4, BF16NP)

    conv_w = _to_pblocks(p["conv_w"][perm], NBLK_F, np.float32)
    conv_b = _to_pblocks(p["conv_b"][perm][:, None], NBLK_F, np.float32)
    xproj_w = _to_pblocks(p["xproj_w"][perm], NBLK_F, BF16NP)
    dt_w = np.ascontiguousarray(p["dt_w"][:, sl_own].astype(BF16NP))
    dt_b = _to_pblocks(p["dt_b"][sl_own][:, None], NBLK_H, np.float32)
    A = _to_pblocks(-np.exp(p["A_log"][sl_own]), NBLK_H, np.float32)
    D = _to_pblocks(p["D_skip"][sl_own][:, None], NBLK_H, np.float32)

    fuse_half = fuse_w[:D_MODEL] if dr == 0 else fuse_w[D_MODEL:]
    w_out_full = p["out_w"].astype(np.float64) @ fuse_half.astype(np.float64)
    w_out = _to_pblocks(w_out_full[sl_own].astype(np.float32), 4, BF16NP)

    return {
        "xT": xT, "w_in": w_in, "conv_w": conv_w, "conv_b": conv_b,
        "xproj_w": xproj_w, "dt_w": dt_w, "dt_b": dt_b, "A": A, "D": D,
        "w_out": w_out,
    }


def kernel(_spmd_kwargs=None, **inputs):
    hs = np.asarray(inputs["hidden_states"], dtype=np.float32)
    fuse_w = np.asarray(inputs["fuse_w"], dtype=np.float32)
    fuse_b = np.asarray(inputs["fuse_b"], dtype=np.float32)
    params = []
    for pre in ("fwd_", "bwd_"):
        params.append({k[len(pre):]: np.asarray(v, dtype=np.float32)
                       for k, v in inputs.items() if k.startswith(pre)})

    nc = _get_program()

    in_maps = []
    core_cfg = []
    prep_cache = {}
    for c in range(8):
        b, dr, h = c >> 2, (c >> 1) & 1, c & 1
        core_cfg.append((b, dr, h))
        key = (b, dr, h)
        if key not in prep_cache:
            prep_cache[key] = _core_inputs(hs, params, fuse_w, b, dr, h)
        in_maps.append(prep_cache[key])

    res = run_bass_kernel_spmd(nc, in_maps, core_ids=list(range(8)),
                               **(_spmd_kwargs or {}))

    out = np.zeros((B, L, D_MODEL), dtype=np.float32)
    for c in range(8):
        b, dr, h = core_cfg[c]
        contrib = (res.results[c]["out_partA"]
                   + res.results[c]["out_partB"]).T  # (L, D_MODEL)
        if dr == 1:
            contrib = contrib[::-1]
        out[b] += contrib
    out += fuse_b[None, None, :]
    if _spmd_kwargs is not None:
        kernel._last_result = res
    return out
